# revision 1
# baseline (speedup 1.0000x reference)
"""CTC loss on 8 Trainium2 cores.

Sharding: pure data parallel, batch 32 -> 4 samples per core.

Device algorithm (per core, SPMD):
  - norm: stream log_probs [4,1600,1024] in [128,1024] tiles; per-t logsumexp
    via reduce_max + ACT Exp(bias=-max, accum_out) + Ln; masked partial sums
    accumulated into a [128,4] tile (host finishes the t-sum in f64).
  - trellis: wavefront decomposition of the CTC forward recurrence.
    Partition p = b*32 + c where c indexes NT=32 time-chunks of Tc=50 steps.
    Cell (s, c) = state s's alpha series over chunk c, computed at wavefront
    w = s + c via one tensor_tensor_scan (state = e*state + b along t).
    Neighbor series (s-1,c), (s-2,c) live at wavefronts w-1, w-2 on the SAME
    partition; each cell's series is stored with its initial value prepended
    (length Tc+1), so the t-1-shifted neighbor series is just cols [0:Tc].
    The chunk-carry (s,c-1) -> (s,c) initial crosses one partition via
    stream_shuffle. Per-cell log-scales Lall[p,w] keep everything in fp32
    range; scale ratios are applied via per-partition scalars (STT ops).
  Host: exact emission gather/scaling tables (compensated by Cb), final
  readout of the two terminal states' cells, loss = norm - llh.
"""
import os
import numpy as np

B, T, C, L = 32, 1600, 1024, 128
S = 2 * L + 1            # 257
Tc, NT = 50, 32          # chunk length, chunks (= partitions per sample)
W = S + NT - 1           # 288 wavefronts
PAD = 3                  # zero wavefront slots before w=0
NCORES = 8
BPC = B // NCORES        # 4 samples per core
NTILE = (T + 127) // 128  # 13 norm tiles per sample (last has 64 rows)
EPS = 1e-30
DCLAMP = 45.0
ANCH = 8.0
AXW = (W + PAD) * (Tc + 1)   # AX columns

_CACHE = {}


def _build_program():
    import concourse.bass as bass
    import concourse.bacc as bacc
    import concourse.mybir as mybir
    from concourse.tile import TileContext

    dt = mybir.dt.float32
    Alu = mybir.AluOpType
    Act = mybir.ActivationFunctionType
    X = mybir.AxisListType.X

    nc = bacc.Bacc("TRN2", target_bir_lowering=False, debug=False,
                   num_devices=NCORES)

    lp_in = nc.dram_tensor("lp_in", [BPC, T, C], dt, kind="ExternalInput")
    ew_in = nc.dram_tensor("ew_in", [128, W * Tc], dt, kind="ExternalInput")
    kl_in = nc.dram_tensor("kl_in", [128, W], dt, kind="ExternalInput")
    cm_in = nc.dram_tensor("cm_in", [128, 2], dt, kind="ExternalInput")
    tm_in = nc.dram_tensor("tm_in", [128, BPC * NTILE], dt, kind="ExternalInput")
    ax_out = nc.dram_tensor("ax_out", [128, AXW], dt, kind="ExternalOutput")
    ll_out = nc.dram_tensor("ll_out", [128, W + PAD], dt, kind="ExternalOutput")
    na_out = nc.dram_tensor("na_out", [128, BPC], dt, kind="ExternalOutput")

    rot1 = [(i - 1) % 32 for i in range(32)]

    with TileContext(nc) as tc:
        with (
            tc.tile_pool(name="big", bufs=1) as big,
            tc.tile_pool(name="lp", bufs=3) as lppool,
            tc.tile_pool(name="scr", bufs=1) as scr,
            tc.tile_pool(name="st", bufs=2) as st,
        ):
            AX = big.tile([128, AXW], dt)
            EW = big.tile([128, W * Tc], dt)
            LLt = big.tile([128, W + PAD], dt)
            KL = big.tile([128, W], dt)
            CM = big.tile([128, 2], dt)
            TM = big.tile([128, BPC * NTILE], dt)
            ACC = big.tile([128, BPC], dt)

            nc.gpsimd.dma_start(EW[:], ew_in[:])
            nc.gpsimd.dma_start(KL[:], kl_in[:])
            nc.gpsimd.dma_start(CM[:], cm_in[:])
            nc.gpsimd.dma_start(TM[:], tm_in[:])
            nc.vector.memset(AX[:, 0:PAD * (Tc + 1)], 0.0)
            nc.vector.memset(LLt[:, 0:PAD], 0.0)
            nc.vector.memset(ACC[:], 0.0)

            # ---------------- norm phase (interleaved by Tile) -------------
            exp_scr = scr.tile([128, C], dt)
            for b in range(BPC):
                for k in range(NTILE):
                    t0 = k * 128
                    rows = min(128, T - t0)
                    lt = lppool.tile([128, C], dt, tag="lp")
                    nc.gpsimd.dma_start(lt[:rows, :], lp_in[b, t0:t0 + rows, :])
                    nmx = st.tile([128, 1], dt, tag="nmx")
                    nc.vector.tensor_reduce(nmx[:rows], lt[:rows, :], X,
                                            Alu.max, negate=True)
                    sums = st.tile([128, 1], dt, tag="sums")
                    nc.scalar.activation(exp_scr[:rows, :], lt[:rows, :],
                                         Act.Exp, bias=nmx[:rows],
                                         scale=1.0, accum_out=sums[:rows])
                    lg = st.tile([128, 1], dt, tag="lg")
                    nc.scalar.activation(lg[:rows], sums[:rows], Act.Ln)
                    ctr = st.tile([128, 1], dt, tag="ctr")
                    # lse = ln(sum) - (-max)
                    nc.vector.tensor_tensor(out=ctr[:rows], in0=lg[:rows],
                                            in1=nmx[:rows], op=Alu.subtract)
                    col = b * NTILE + k
                    nc.vector.tensor_tensor(out=ctr[:rows], in0=ctr[:rows],
                                            in1=TM[:rows, col:col + 1],
                                            op=Alu.mult)
                    nc.vector.tensor_tensor(out=ACC[:rows, b:b + 1],
                                            in0=ACC[:rows, b:b + 1],
                                            in1=ctr[:rows], op=Alu.add)
            nc.gpsimd.dma_start(na_out[:], ACC[:])

            # ---------------- wavefront scan -------------------------------
            for w in range(W):
                wi = w + PAD
                b0 = wi * (Tc + 1)
                b1 = (wi - 1) * (Tc + 1)
                b2 = (wi - 2) * (Tc + 1)

                shv = st.tile([128, 1], dt, tag="shv")
                sl1 = st.tile([128, 1], dt, tag="sl1")
                nc.vector.stream_shuffle(shv[:], AX[:, b1 + Tc:b1 + Tc + 1], rot1)
                nc.vector.stream_shuffle(sl1[:], LLt[:, wi - 1:wi], rot1)

                LNS = st.tile([128, 3], dt, tag="LNS")
                nc.vector.tensor_scalar_max(LNS[:, 0:1], shv[:], EPS)
                nc.vector.tensor_scalar_max(LNS[:, 1:2],
                                            AX[:, b1 + Tc:b1 + Tc + 1], EPS)
                nc.vector.tensor_scalar_max(LNS[:, 2:3],
                                            AX[:, b2 + Tc:b2 + Tc + 1], EPS)
                LNV = st.tile([128, 3], dt, tag="LNV")
                nc.scalar.activation(LNV[:], LNS[:], Act.Ln)

                TK = st.tile([128, 1], dt, tag="TK")
                nc.vector.tensor_tensor(out=TK[:], in0=LLt[:, wi - 2:wi - 1],
                                        in1=KL[:, w:w + 1], op=Alu.add)
                LS = st.tile([128, 3], dt, tag="LS")
                nc.vector.tensor_tensor(out=LS[:, 0:1], in0=sl1[:],
                                        in1=LNV[:, 0:1], op=Alu.add)
                nc.vector.scalar_tensor_tensor(
                    out=LS[:, 1:2], in0=LNV[:, 1:2], scalar=-ANCH,
                    in1=LLt[:, wi - 1:wi], op0=Alu.add, op1=Alu.add)
                nc.vector.scalar_tensor_tensor(
                    out=LS[:, 2:3], in0=LNV[:, 2:3], scalar=-ANCH,
                    in1=TK[:], op0=Alu.add, op1=Alu.add)
                Lnew = st.tile([128, 1], dt, tag="Lnew")
                nc.vector.tensor_reduce(Lnew[:], LS[:], X, Alu.max)
                nc.vector.tensor_tensor(out=Lnew[:], in0=Lnew[:],
                                        in1=CM[:, 0:1], op=Alu.mult)

                D = st.tile([128, 3], dt, tag="D")
                nc.vector.tensor_tensor(out=D[:, 0:1], in0=LLt[:, wi - 1:wi],
                                        in1=Lnew[:], op=Alu.subtract)
                nc.vector.tensor_tensor(out=D[:, 1:2], in0=TK[:],
                                        in1=Lnew[:], op=Alu.subtract)
                nc.vector.tensor_tensor(out=D[:, 2:3], in0=LS[:, 0:1],
                                        in1=Lnew[:], op=Alu.subtract)
                nc.vector.tensor_scalar_min(D[:], D[:], DCLAMP)
                EX = st.tile([128, 3], dt, tag="EX")
                nc.scalar.activation(EX[:], D[:], Act.Exp)

                alive = st.tile([128, 1], dt, tag="alive")
                nc.vector.tensor_scalar(alive[:], shv[:], EPS, None, Alu.is_gt)
                nc.vector.tensor_tensor(out=alive[:], in0=alive[:],
                                        in1=CM[:, 0:1], op=Alu.mult)
                initc = st.tile([128, 1], dt, tag="initc")
                nc.vector.tensor_tensor(out=initc[:], in0=EX[:, 2:3],
                                        in1=alive[:], op=Alu.mult)
                if w == 0:
                    nc.vector.tensor_tensor(out=initc[:], in0=initc[:],
                                            in1=CM[:, 1:2], op=Alu.add)

                P1 = st.tile([128, Tc], dt, tag="P1")
                nc.vector.tensor_scalar_mul(P1[:], AX[:, b1:b1 + Tc], EX[:, 0:1])
                U = st.tile([128, Tc], dt, tag="U")
                nc.vector.scalar_tensor_tensor(
                    out=U[:], in0=AX[:, b2:b2 + Tc], scalar=EX[:, 1:2],
                    in1=P1[:], op0=Alu.mult, op1=Alu.add)
                BS = st.tile([128, Tc], dt, tag="BS")
                ew_sl = EW[:, w * Tc:(w + 1) * Tc]
                nc.vector.tensor_tensor(out=BS[:], in0=U[:], in1=ew_sl,
                                        op=Alu.mult)
                nc.vector.tensor_tensor_scan(
                    out=AX[:, b0 + 1:b0 + 1 + Tc], data0=ew_sl, data1=BS[:],
                    initial=initc[:], op0=Alu.mult, op1=Alu.add)
                nc.vector.tensor_copy(AX[:, b0:b0 + 1], initc[:])
                nc.vector.tensor_copy(LLt[:, wi:wi + 1], Lnew[:])

            nc.gpsimd.dma_start(ax_out[:], AX[:])
            nc.gpsimd.dma_start(ll_out[:], LLt[:])

    nc.compile()
    return nc


def _host_prep_core(lp_c, tgt_c, il_c, tl_c):
    """Build per-core input tensors. lp_c: [BPC,T,C] f32."""
    ew = np.zeros((128, W * Tc), np.float32)
    kl = np.full((128, W), -1e30, np.float32)
    cm = np.zeros((128, 2), np.float32)
    tm = np.zeros((128, BPC * NTILE), np.float32)
    meta = []
    for b in range(BPC):
        il = int(il_c[b]); tl = int(tl_c[b])
        Sb = 2 * tl + 1
        ext = np.zeros(S, np.int64); ext[1::2] = tgt_c[b]
        skip = np.zeros(S, bool); skip[3::2] = (tgt_c[b, 1:] != tgt_c[b, :-1])
        E = lp_c[b][:, ext].astype(np.float64)            # [T,S]
        # c_t = reachable-band max - 2
        tt = np.arange(il)
        lo = np.maximum(0, Sb - 1 - 2 * (il - 1 - tt))
        hi = np.minimum(Sb - 1, 2 * tt + 1)
        sidx = np.arange(S)[None, :]
        bandmask = (sidx >= lo[:, None]) & (sidx <= hi[:, None])
        c = np.where(bandmask, E[:il], -np.inf).max(axis=1) - 2.0
        eh = np.zeros((T, S), np.float32)
        eh[:il, :Sb] = np.exp(E[:il, :Sb] - c[:, None]).astype(np.float32)
        Cb = float(c.sum())
        # skewed tables: partition p = b*32 + c_chunk
        for cc in range(NT):
            p = b * 32 + cc
            chunk = eh[cc * Tc:(cc + 1) * Tc, :]           # [Tc, S]
            blk = ew[p].reshape(W, Tc)
            blk[cc:cc + S, :] = chunk.T
            kl[p, cc:cc + S][skip] = 0.0
        cm[b * 32 + 1:(b + 1) * 32, 0] = 1.0              # CMASK: 0 for c=0
        cm[b * 32, 1] = 1.0                               # INIT0 col
        for k in range(NTILE):
            t0 = k * 128
            rows = min(128, T - t0)
            tcol = t0 + np.arange(rows)
            tm[:rows, b * NTILE + k] = (tcol < il).astype(np.float32)
        meta.append((il, tl, Sb, Cb))
    return ew, kl, cm, tm, meta


def kernel(log_probs, targets, input_lengths, target_lengths):
    from concourse.bass_utils import run_bass_kernel_spmd

    lp = np.ascontiguousarray(np.asarray(log_probs, dtype=np.float32))
    tgt = np.asarray(targets)
    il = np.asarray(input_lengths).astype(np.int64)
    tl = np.asarray(target_lengths).astype(np.int64)

    if "nc" not in _CACHE:
        _CACHE["nc"] = _build_program()
    nc = _CACHE["nc"]

    in_maps = []
    metas = []
    for core in range(NCORES):
        sl = slice(core * BPC, (core + 1) * BPC)
        ew, kl, cm, tm, meta = _host_prep_core(lp[sl], tgt[sl], il[sl], tl[sl])
        in_maps.append({"lp_in": lp[sl], "ew_in": ew, "kl_in": kl,
                        "cm_in": cm, "tm_in": tm})
        metas.append(meta)

    trace = bool(os.environ.get("CTC_BASS_TRACE"))
    res = run_bass_kernel_spmd(nc, in_maps, list(range(NCORES)), trace=trace)
    if trace:
        print(f"HW exec time: {res.exec_time_ns} ns")

    losses = np.zeros(B, np.float64)
    for core in range(NCORES):
        axo = res.results[core]["ax_out"]
        llo = res.results[core]["ll_out"]
        nao = res.results[core]["na_out"].astype(np.float64)
        for b in range(BPC):
            il_b, tl_b, Sb, Cb = metas[core][b]
            cstar = (il_b - 1) // Tc
            tau = (il_b - 1) % Tc
            p = b * 32 + cstar
            vals = []
            for s in (Sb - 1, Sb - 2):
                wi = s + cstar + PAD
                v = np.float64(axo[p, wi * (Tc + 1) + 1 + tau])
                lam = np.float64(llo[p, wi])
                vals.append((v, lam))
            mlam = max(v[1] for v in vals)
            tot = sum(v[0] * np.exp(v[1] - mlam) for v in vals)
            llh = np.log(tot) + mlam + Cb
            norm = nao[:, b].sum()
            losses[core * BPC + b] = norm - llh
    return losses.astype(np.float32)


# revision 2
# speedup vs baseline: 1.3913x; 1.3913x over previous
"""CTC loss on 8 Trainium2 cores.

Sharding: pure data parallel, batch 32 -> 4 samples per core.

Device algorithm (per core, SPMD):
  - norm: stream log_probs [4,1600,1024] in [128,1024] tiles; per-t logsumexp
    via reduce_max + ACT Exp(bias=-max, accum_out) + Ln; masked partial sums
    accumulated into a [128,4] tile (host finishes the t-sum in f64).
  - trellis: wavefront decomposition of the CTC forward recurrence in the
    probability domain.  Partition p = b*32 + c, c indexing NT=32 time-chunks
    of Tc=50 steps.  Cell (s, c) = state s's alpha series over chunk c,
    computed at wavefront w = s + c by one tensor_tensor_scan
    (state = e*state + b along t).  Neighbor series (s-1,c), (s-2,c) live at
    wavefronts w-1, w-2 on the SAME partition; each cell's series is stored
    with its initial value prepended (length Tc+1), so the t-1-shifted
    neighbor series is just cols [0:Tc].  The chunk-carry (s,c-1) -> (s,c)
    initial crosses one partition via stream_shuffle.  Per-cell scales are
    EXACT powers of two (exponent-bit extraction; no transcendentals on the
    trellis path), tracked as integer counts NL[p,w]; ratios are applied via
    per-partition scalars.
  Host: exact emission gather/scaling tables (compensated by Cb in f64),
  final readout of the two terminal states' cells, loss = norm - llh.
"""
import os
import numpy as np

B, T, C, L = 32, 1600, 1024, 128
S = 2 * L + 1            # 257
Tc, NT = 50, 32          # chunk length, chunks (= partitions per sample)
W = S + NT - 1           # 288 wavefronts
PAD = 3                  # zero wavefront slots before w=0
NCORES = 8
BPC = B // NCORES        # 4 samples per core
NTILE = (T + 127) // 128  # 13 norm tiles per sample (last has 64 rows)
AXW = (W + PAD) * (Tc + 1)   # AX columns
KNEG = -float(2 ** 26)   # "minus infinity" in exponent-count space

_CACHE = {}


def _build_program():
    import concourse.bacc as bacc
    import concourse.mybir as mybir
    from concourse.tile import TileContext

    dt = mybir.dt.float32
    di = mybir.dt.int32
    Alu = mybir.AluOpType
    Act = mybir.ActivationFunctionType
    X = mybir.AxisListType.X

    nc = bacc.Bacc("TRN2", target_bir_lowering=False, debug=False,
                   num_devices=NCORES)

    lp_in = nc.dram_tensor("lp_in", [BPC, T, C], dt, kind="ExternalInput")
    ew_in = nc.dram_tensor("ew_in", [128, W * Tc], dt, kind="ExternalInput")
    kn_in = nc.dram_tensor("kn_in", [128, W], dt, kind="ExternalInput")
    k01_in = nc.dram_tensor("k01_in", [128, W], dt, kind="ExternalInput")
    cm_in = nc.dram_tensor("cm_in", [128, 2], dt, kind="ExternalInput")
    tm_in = nc.dram_tensor("tm_in", [128, BPC * NTILE], dt, kind="ExternalInput")
    ax_out = nc.dram_tensor("ax_out", [128, AXW], dt, kind="ExternalOutput")
    ll_out = nc.dram_tensor("ll_out", [128, W + PAD], dt, kind="ExternalOutput")
    na_out = nc.dram_tensor("na_out", [128, BPC], dt, kind="ExternalOutput")

    rot1 = [(i - 1) % 32 for i in range(32)]

    with TileContext(nc) as tc:
        with (
            tc.tile_pool(name="big", bufs=1) as big,
            tc.tile_pool(name="lp", bufs=3) as lppool,
            tc.tile_pool(name="scr", bufs=1) as scr,
            tc.tile_pool(name="st", bufs=2) as st,
        ):
            AX = big.tile([128, AXW], dt)
            EW = big.tile([128, W * Tc], dt)
            NL = big.tile([128, W + PAD], dt)
            KN = big.tile([128, W], dt)
            K01 = big.tile([128, W], dt)
            CM = big.tile([128, 2], dt)
            TM = big.tile([128, BPC * NTILE], dt)
            ACC = big.tile([128, BPC], dt)

            nc.gpsimd.dma_start(EW[:], ew_in[:])
            nc.gpsimd.dma_start(KN[:], kn_in[:])
            nc.gpsimd.dma_start(K01[:], k01_in[:])
            nc.gpsimd.dma_start(CM[:], cm_in[:])
            nc.gpsimd.dma_start(TM[:], tm_in[:])
            nc.vector.memset(AX[:, 0:PAD * (Tc + 1)], 0.0)
            nc.vector.memset(NL[:, 0:PAD], 0.0)
            nc.vector.memset(ACC[:], 0.0)

            # ---------------- norm phase (interleaved by Tile) -------------
            exp_scr = scr.tile([128, C], dt)
            for b in range(BPC):
                for k in range(NTILE):
                    t0 = k * 128
                    rows = min(128, T - t0)
                    lt = lppool.tile([128, C], dt, tag="lp")
                    nc.gpsimd.dma_start(lt[:rows, :], lp_in[b, t0:t0 + rows, :])
                    nmx = st.tile([128, 1], dt, tag="nmx")
                    nc.vector.tensor_reduce(nmx[:rows], lt[:rows, :], X,
                                            Alu.max, negate=True)
                    sums = st.tile([128, 1], dt, tag="sums")
                    nc.scalar.activation(exp_scr[:rows, :], lt[:rows, :],
                                         Act.Exp, bias=nmx[:rows],
                                         scale=1.0, accum_out=sums[:rows])
                    lg = st.tile([128, 1], dt, tag="lg")
                    nc.scalar.activation(lg[:rows], sums[:rows], Act.Ln)
                    ctr = st.tile([128, 1], dt, tag="ctr")
                    # lse = ln(sum) - (-max)
                    nc.vector.tensor_tensor(out=ctr[:rows], in0=lg[:rows],
                                            in1=nmx[:rows], op=Alu.subtract)
                    col = b * NTILE + k
                    nc.vector.tensor_tensor(out=ctr[:rows], in0=ctr[:rows],
                                            in1=TM[:rows, col:col + 1],
                                            op=Alu.mult)
                    nc.vector.tensor_tensor(out=ACC[:rows, b:b + 1],
                                            in0=ACC[:rows, b:b + 1],
                                            in1=ctr[:rows], op=Alu.add)
            nc.gpsimd.dma_start(na_out[:], ACC[:])

            # ---------------- wavefront scan (pow2 scales) ------------------
            for w in range(W):
                wi = w + PAD
                b0 = wi * (Tc + 1)
                b1 = (wi - 1) * (Tc + 1)
                b2 = (wi - 2) * (Tc + 1)

                # VB: [s_iv, p1e, p2e] end-values; NST: [Nsl1, Nw1, NTK]
                VB = st.tile([128, 3], dt, tag="VB")
                nc.vector.stream_shuffle(VB[:, 0:1],
                                         AX[:, b1 + Tc:b1 + Tc + 1], rot1)
                nc.vector.tensor_copy(VB[:, 1:2], AX[:, b1 + Tc:b1 + Tc + 1])
                nc.vector.tensor_copy(VB[:, 2:3], AX[:, b2 + Tc:b2 + Tc + 1])
                NST = st.tile([128, 3], dt, tag="NST")
                nc.vector.stream_shuffle(NST[:, 0:1], NL[:, wi - 1:wi], rot1)
                nc.vector.tensor_copy(NST[:, 1:2], NL[:, wi - 1:wi])
                nc.vector.tensor_tensor(out=NST[:, 2:3],
                                        in0=NL[:, wi - 2:wi - 1],
                                        in1=KN[:, w:w + 1], op=Alu.add)

                # exponents of the three end-values (0 for zero/denormal)
                EI = st.tile([128, 3], di, tag="EI")
                nc.vector.tensor_scalar(EI[:], VB[:].bitcast(di), 23, None,
                                        Alu.logical_shift_right)
                EF = st.tile([128, 3], dt, tag="EF")
                nc.vector.tensor_copy(EF[:], EI[:])

                CAND = st.tile([128, 3], dt, tag="CAND")
                nc.vector.scalar_tensor_tensor(
                    out=CAND[:, 0:1], in0=EF[:, 0:1], scalar=-127.0,
                    in1=NST[:, 0:1], op0=Alu.add, op1=Alu.add)
                nc.vector.scalar_tensor_tensor(
                    out=CAND[:, 1:2], in0=EF[:, 1:2], scalar=-139.0,
                    in1=NST[:, 1:2], op0=Alu.add, op1=Alu.add)
                nc.vector.scalar_tensor_tensor(
                    out=CAND[:, 2:3], in0=EF[:, 2:3], scalar=-139.0,
                    in1=NST[:, 2:3], op0=Alu.add, op1=Alu.add)
                Nnew = st.tile([128, 1], dt, tag="Nnew")
                nc.vector.tensor_reduce(Nnew[:], CAND[:], X, Alu.max)
                nc.vector.tensor_tensor(out=Nnew[:], in0=Nnew[:],
                                        in1=CM[:, 0:1], op=Alu.mult)

                NN = st.tile([128, 1], dt, tag="NN")
                nc.vector.tensor_scalar_mul(NN[:], Nnew[:], -1.0)
                DN = st.tile([128, 3], dt, tag="DN")
                nc.vector.tensor_scalar(DN[:], NST[:], NN[:], None, Alu.add)
                nc.vector.tensor_scalar_max(DN[:], DN[:], -126.0)
                nc.vector.tensor_scalar_min(DN[:], DN[:], 126.0)
                # 2^DN: (DN+127) -> int -> <<23 -> bitcast
                PF = st.tile([128, 3], dt, tag="PF")
                nc.vector.tensor_scalar_add(PF[:], DN[:], 127.0)
                PI = st.tile([128, 3], di, tag="PI")
                nc.vector.tensor_copy(PI[:], PF[:])
                PS = st.tile([128, 3], di, tag="PS")
                nc.vector.tensor_scalar(PS[:], PI[:], 23, None,
                                        Alu.logical_shift_left)
                PW = PS[:].bitcast(dt)   # [128,3]: 2^{Nsl1-Nnew}, R1, Q2raw

                initc = st.tile([128, 1], dt, tag="initc")
                nc.vector.tensor_tensor(out=initc[:], in0=VB[:, 0:1],
                                        in1=PW[:, 0:1], op=Alu.mult)
                nc.vector.tensor_tensor(out=initc[:], in0=initc[:],
                                        in1=CM[:, 0:1], op=Alu.mult)
                if w == 0:
                    nc.vector.tensor_tensor(out=initc[:], in0=initc[:],
                                            in1=CM[:, 1:2], op=Alu.add)
                Q2x = st.tile([128, 1], dt, tag="Q2x")
                nc.vector.tensor_tensor(out=Q2x[:], in0=PW[:, 2:3],
                                        in1=K01[:, w:w + 1], op=Alu.mult)

                P1 = st.tile([128, Tc], dt, tag="P1")
                nc.vector.tensor_scalar_mul(P1[:], AX[:, b1:b1 + Tc],
                                            PW[:, 1:2])
                U = st.tile([128, Tc], dt, tag="U")
                nc.vector.scalar_tensor_tensor(
                    out=U[:], in0=AX[:, b2:b2 + Tc], scalar=Q2x[:],
                    in1=P1[:], op0=Alu.mult, op1=Alu.add)
                BS = st.tile([128, Tc], dt, tag="BS")
                ew_sl = EW[:, w * Tc:(w + 1) * Tc]
                nc.vector.tensor_tensor(out=BS[:], in0=U[:], in1=ew_sl,
                                        op=Alu.mult)
                nc.vector.tensor_tensor_scan(
                    out=AX[:, b0 + 1:b0 + 1 + Tc], data0=ew_sl, data1=BS[:],
                    initial=initc[:], op0=Alu.mult, op1=Alu.add)
                nc.vector.tensor_copy(AX[:, b0:b0 + 1], initc[:])
                nc.vector.tensor_copy(NL[:, wi:wi + 1], Nnew[:])

            nc.gpsimd.dma_start(ax_out[:], AX[:])
            nc.gpsimd.dma_start(ll_out[:], NL[:])

    nc.compile()
    return nc


def _host_prep_core(lp_c, tgt_c, il_c, tl_c):
    """Build per-core input tensors. lp_c: [BPC,T,C] f32."""
    ew = np.zeros((128, W * Tc), np.float32)
    kn = np.full((128, W), KNEG, np.float32)
    k01 = np.zeros((128, W), np.float32)
    cm = np.zeros((128, 2), np.float32)
    tm = np.zeros((128, BPC * NTILE), np.float32)
    meta = []
    for b in range(BPC):
        il = int(il_c[b]); tl = int(tl_c[b])
        Sb = 2 * tl + 1
        ext = np.zeros(S, np.int64); ext[1::2] = tgt_c[b]
        skip = np.zeros(S, bool); skip[3::2] = (tgt_c[b, 1:] != tgt_c[b, :-1])
        E = lp_c[b][:, ext].astype(np.float64)            # [T,S]
        # c_t = reachable-band max - 2
        tt = np.arange(il)
        lo = np.maximum(0, Sb - 1 - 2 * (il - 1 - tt))
        hi = np.minimum(Sb - 1, 2 * tt + 1)
        sidx = np.arange(S)[None, :]
        bandmask = (sidx >= lo[:, None]) & (sidx <= hi[:, None])
        c = np.where(bandmask, E[:il], -np.inf).max(axis=1) - 2.0
        eh = np.zeros((T, S), np.float32)
        eh[:il, :Sb] = np.exp(E[:il, :Sb] - c[:, None]).astype(np.float32)
        Cb = float(c.sum())
        # skewed tables: partition p = b*32 + c_chunk
        for cc in range(NT):
            p = b * 32 + cc
            chunk = eh[cc * Tc:(cc + 1) * Tc, :]           # [Tc, S]
            blk = ew[p].reshape(W, Tc)
            blk[cc:cc + S, :] = chunk.T
            kn[p, cc:cc + S][skip] = 0.0
            k01[p, cc:cc + S][skip] = 1.0
        cm[b * 32 + 1:(b + 1) * 32, 0] = 1.0              # CMASK: 0 for c=0
        cm[b * 32, 1] = 1.0                               # INIT0 col
        for k in range(NTILE):
            t0 = k * 128
            rows = min(128, T - t0)
            tcol = t0 + np.arange(rows)
            tm[:rows, b * NTILE + k] = (tcol < il).astype(np.float32)
        meta.append((il, tl, Sb, Cb))
    return ew, kn, k01, cm, tm, meta


def kernel(log_probs, targets, input_lengths, target_lengths):
    from concourse.bass_utils import run_bass_kernel_spmd

    lp = np.ascontiguousarray(np.asarray(log_probs, dtype=np.float32))
    tgt = np.asarray(targets)
    il = np.asarray(input_lengths).astype(np.int64)
    tl = np.asarray(target_lengths).astype(np.int64)

    if "nc" not in _CACHE:
        _CACHE["nc"] = _build_program()
    nc = _CACHE["nc"]

    in_maps = []
    metas = []
    for core in range(NCORES):
        sl = slice(core * BPC, (core + 1) * BPC)
        ew, kn, k01, cm, tm, meta = _host_prep_core(lp[sl], tgt[sl],
                                                    il[sl], tl[sl])
        in_maps.append({"lp_in": lp[sl], "ew_in": ew, "kn_in": kn,
                        "k01_in": k01, "cm_in": cm, "tm_in": tm})
        metas.append(meta)

    trace = bool(os.environ.get("CTC_BASS_TRACE"))
    res = run_bass_kernel_spmd(nc, in_maps, list(range(NCORES)), trace=trace)
    if trace:
        print(f"HW exec time: {res.exec_time_ns} ns")

    LN2 = np.log(2.0)
    losses = np.zeros(B, np.float64)
    for core in range(NCORES):
        axo = res.results[core]["ax_out"]
        llo = res.results[core]["ll_out"]
        nao = res.results[core]["na_out"].astype(np.float64)
        for b in range(BPC):
            il_b, tl_b, Sb, Cb = metas[core][b]
            cstar = (il_b - 1) // Tc
            tau = (il_b - 1) % Tc
            p = b * 32 + cstar
            vals = []
            for s in (Sb - 1, Sb - 2):
                wi = s + cstar + PAD
                v = np.float64(axo[p, wi * (Tc + 1) + 1 + tau])
                lam = np.float64(llo[p, wi]) * LN2
                vals.append((v, lam))
            mlam = max(v[1] for v in vals)
            tot = sum(v[0] * np.exp(v[1] - mlam) for v in vals)
            llh = np.log(tot) + mlam + Cb
            norm = nao[:, b].sum()
            losses[core * BPC + b] = norm - llh
    return losses.astype(np.float32)
